# revision 9
# baseline (speedup 1.0000x reference)
"""Bilateral anti-alias filter on Trainium2, 8-core data parallel.

Full inputs: images [16,3,512,512] f32, spatial_kernel [5,5] f32.
Shards the batch over 8 NeuronCores (2 images each), runs a Bass/Tile
kernel per core, gathers the full output.

Math (per pixel, K=5, sigma_i=0.1):
  w_t = s_t * exp(-50*(p_t - c)^2),  out = sum(w_t*p_t)/(sum(w_t)+eps)
Pair symmetry: w for tap -(di,dj) at pixel r equals w for tap +(di,dj)
at pixel r-(di,dj), so only 12 weight planes are computed.

U-trick: with d_v = p(.+v) - p and U_v = w_v*d_v,
  num = p*den + sum_v [U_v(r) - U_v(r-v)]   =>   out = p + pu/(1+pw)
so per pair only ONE product (U) is needed on the Vector engine; the
+/-1 signs and row shifts are folded into 0/+-1 matrices fed to the
TensorEngine, which accumulates pw (denominator) and pu (numerator
correction) into PSUM.  Column shifts are SBUF slices (free).

Engine budget per 128-row band tile (~30us): DVE ~30us (subs, U, 2
group squares, casts, finalize), ACT ~30us (4 group squares, exps,
Ln/Exp reciprocal; one table set, preloaded), PE ~132 matmuls.
GpSimd is kept idle: it shares an exclusive SBUF port pair with the
DVE's second read port, so any GpSimd op blocks every tensor_tensor.
"""
import sys

sys.path.insert(0, "/opt/trn_rl_repo")

import numpy as np
import ml_dtypes
from contextlib import ExitStack

import concourse.bass as bass
import concourse.tile as tile
from concourse import bacc, mybir
from concourse.bass_utils import run_bass_kernel_spmd

f32 = mybir.dt.float32
bf16 = mybir.dt.bfloat16
AF = mybir.ActivationFunctionType
Alu = mybir.AluOpType

N_CORES = 8
B_FULL, C, H, W = 16, 3, 512, 512
B_SH = B_FULL // N_CORES  # 2 images per core
KK = 5
PAD = KK // 2  # 2
INV2SIG2 = 1.0 / (2.0 * 0.1 * 0.1)  # 50.0

# 12 pairs (di, dj) with di >= 0, lexicographically positive
PAIRS = [
    (0, 1), (0, 2),
    (1, -2), (1, -1), (1, 0), (1, 1), (1, 2),
    (2, -2), (2, -1), (2, 0), (2, 1), (2, 2),
]

# groups pair taps with EQUAL spatial weight (same di^2+dj^2) so one
# exp bias serves the whole group
GROUPS = [
    [(0, 1), (1, 0)], [(1, -1), (1, 1)], [(0, 2), (2, 0)],
    [(1, -2), (1, 2)], [(2, -1), (2, 1)], [(2, -2), (2, 2)],
]
SQ_DVE_GROUPS = {1, 4}  # these groups square on DVE; rest on ACT
NOUT = 124  # output rows per tile
NG = 128    # grid partitions (= NOUT + 4)
WB = W + 4  # 516: padded col buffer, idx j <-> col j-2

# shift-matrix variant indices (lhsT slices of the shifts tensor)
SH_S0, SH_S1, SH_S2 = 0, 1, 2
SH_S2pS1, SH_S2pS0 = 3, 4          # pw merged dj=0 (di=1, di=2)
SH_nS0, SH_nS1, SH_nS2 = 5, 6, 7   # negated, for pu shifted terms
SH_S2mS1, SH_S2mS0 = 8, 9          # pu merged dj=0
N_SHIFT = 10


def _row_bands(h):
    """Tile start rows: uniform NOUT-row bands; last band overlaps upward."""
    bands = list(range(0, h - NOUT + 1, NOUT))
    if bands[-1] != h - NOUT:
        bands.append(h - NOUT)
    return bands


def _reflect_runs(v0, v1, h):
    """Split virtual row range [v0, v1] into runs of physical rows.
    Returns list of (p_offset, phys_start, count, step) with step +-1."""
    runs = []
    v = v0
    while v <= v1:
        if v < 0:
            e = min(-1, v1)
            runs.append((v - v0, -v, e - v + 1, -1))
            v = e + 1
        elif v >= h:
            e = v1
            runs.append((v - v0, 2 * h - 2 - v, e - v + 1, -1))
            v = e + 1
        else:
            e = min(h - 1, v1)
            runs.append((v - v0, v, e - v + 1, 1))
            v = e + 1
    return runs


def build_bilateral(nc, biases, h=H, w=W, b_sh=B_SH, c=C):
    """Emit the per-core program into nc (a Bacc). biases[(di,dj)] = ln s."""
    wb = w + 4
    img_d = nc.dram_tensor("images", [b_sh, c, h, w], f32, kind="ExternalInput").ap()
    shifts_d = nc.dram_tensor(
        "shifts", [NG, N_SHIFT, NOUT], bf16, kind="ExternalInput"
    ).ap()
    out_d = nc.dram_tensor("out", [b_sh, c, h, w], f32, kind="ExternalOutput").ap()

    # const APs for activation biases
    for val in sorted(set(biases.values()) | {1.0}):
        key = (f32, val)
        if key not in nc.const_aps.aps:
            t = nc.alloc_sbuf_tensor(f"cbias-{val}", [128, 1], f32)
            nc.gpsimd.memset(t.ap(), val)
            nc.const_aps.aps[key] = t.ap()
    nc.all_engine_barrier()

    # Pre-place one ACT table load for the set containing Square+Exp+Ln;
    # otherwise the compiler's greedy chooser thrashes between
    # exp_and_others and natural_log (2 reloads/tile, ~2.7us each).
    from concourse.hw_specs import get_activation_tables

    set_names = list(get_activation_tables(nc.m.arch))
    nc.scalar.add_instruction(
        mybir.InstLoadActFuncSet(
            name=nc.get_next_instruction_name(),
            act_func_set_id=set_names.index("natural_log_exp_and_others"),
            ins=[],
            outs=[],
        )
    )

    bands = _row_bands(h)

    with tile.TileContext(nc) as tc, ExitStack() as ctx:
        consts = ctx.enter_context(tc.tile_pool(name="consts", bufs=1))
        imgs_f = ctx.enter_context(tc.tile_pool(name="imgs_f", bufs=2))
        imgs_b = ctx.enter_context(tc.tile_pool(name="imgs_b", bufs=2))
        planes = ctx.enter_context(tc.tile_pool(name="planes", bufs=2))
        prods = ctx.enter_context(tc.tile_pool(name="prods", bufs=2))
        finals = ctx.enter_context(tc.tile_pool(name="finals", bufs=2))
        psums = ctx.enter_context(tc.tile_pool(name="psums", bufs=1, space="PSUM"))

        shifts = consts.tile([NG, N_SHIFT, NOUT], bf16)
        nc.sync.dma_start(shifts[:], shifts_d[:])

        def load_ifs(bi, r0):
            """Issue the DMA loads of the 3 row-shifted f32 image copies.
            Emitted one tile ahead so prefetch isn't head-of-line blocked
            behind the previous tile's output write on the sync queue."""
            ifs = []
            for s in range(3):
                t = imgs_f.tile([NG, c, wb], f32, tag=f"i{s}f")
                refl_rows = []
                for (po, ps, cnt, step) in _reflect_runs(
                    r0 - 2 + s, r0 - 2 + s + NG - 1, h
                ):
                    if step == 1:
                        src = img_d[bi, :, ps : ps + cnt, :]
                        nc.sync.dma_start(
                            t[po : po + cnt, :, 2 : 2 + w],
                            src.rearrange("c r n -> r c n"),
                        )
                    else:
                        for k in range(cnt):
                            refl_rows.append((po + k, ps - k))
                # reflect rows duplicate rows already in the tile
                for (p_dst, phys) in refl_rows:
                    p_src = phys - (r0 - 2 + s)
                    nc.sync.dma_start(
                        t[p_dst : p_dst + 1, :, 2 : 2 + w],
                        t[p_src : p_src + 1, :, 2 : 2 + w],
                    )
                ifs.append(t)
            return ifs

        tiles_l = [(bi, r0) for bi in range(b_sh) for r0 in bands]
        pending = load_ifs(*tiles_l[0])
        prev_end = 0
        for t_i, (bi, r0) in enumerate(tiles_l):
            ifs = pending
            if t_i + 1 < len(tiles_l):
                pending = load_ifs(*tiles_l[t_i + 1])
            if True:
                # reflect pad cols: tiny DVE copies (NOT GpSimd: GpSimd
                # ops block the shared SBUF port the DVE TTs need)
                for t in ifs:
                    for (j, jsrc) in ((0, 4), (1, 3), (2 + w, w), (3 + w, w - 1)):
                        nc.vector.tensor_copy(
                            t[:, :, j : j + 1], t[:, :, jsrc : jsrc + 1]
                        )

                # bf16 copies: A (cast), B (A shifted 1 col, via DMA)
                ibA, ibB = [], []
                for s in range(3):
                    a = imgs_b.tile([NG, c, wb], bf16, tag=f"i{s}bA")
                    nc.vector.tensor_copy(a[:], ifs[s][:])
                    ibA.append(a)
                    b = imgs_b.tile([NG, c, wb], bf16, tag=f"i{s}bB")
                    nc.sync.dma_start(b[:, :, 0 : wb - 1], a[:, :, 1:wb])
                    ibB.append(b)

                # ---- PSUM accumulators ----
                pw = psums.tile([NOUT, c, 512], f32, tag="pw")
                pu = psums.tile([NOUT, c, 512], f32, tag="pu")

                n_con = 2 * len(PAIRS) - 2  # pw/pu contributions per channel
                con_i = 0
                for g_i, grp in enumerate(GROUPS):
                    G = len(grp)
                    dg = planes.tile([NG, G * c, w + 2], bf16, tag="d")
                    for gi, (di, dj) in enumerate(grp):
                        cP = -2 if dj > 0 else 0
                        if dj % 2 == 0:
                            dsrc = ibA[di][:, :, cP + dj + 2 : cP + dj + 4 + w]
                        else:
                            dsrc = ibB[di][:, :, cP + dj + 1 : cP + dj + 3 + w]
                        nc.vector.tensor_tensor(
                            dg[:, gi * c : (gi + 1) * c, :], dsrc,
                            ibA[0][:, :, cP + 2 : cP + 4 + w], Alu.subtract,
                        )
                    sqg = planes.tile([NG, G * c, w + 2], bf16, tag="sq")
                    if g_i in SQ_DVE_GROUPS:
                        nc.vector.tensor_tensor(sqg[:], dg[:], dg[:], Alu.mult)
                    else:
                        nc.scalar.activation(sqg[:], dg[:], AF.Square)
                    wg = planes.tile([NG, G * c, w + 2], bf16, tag="w")
                    nc.scalar.activation(
                        wg[:], sqg[:], AF.Exp,
                        bias=biases[grp[0]], scale=-INV2SIG2,
                    )
                    ug = prods.tile([NG, G * c, w + 2], bf16, tag="u")
                    nc.vector.tensor_tensor(ug[:], wg[:], dg[:], Alu.mult)

                    for gi, (di, dj) in enumerate(grp):
                        cP = -2 if dj > 0 else 0
                        wp = wg[:, gi * c : (gi + 1) * c, :]
                        up = ug[:, gi * c : (gi + 1) * c, :]
                        first = con_i == 0
                        if dj == 0:
                            # merged direct+shifted (same rhs window, cP=0)
                            last = con_i == n_con - 1
                            for ch in range(c):
                                nc.tensor.matmul(
                                    pw[:, ch, 0:w],
                                    shifts[:, SH_S2pS1 if di == 1 else SH_S2pS0, :],
                                    wp[:, ch, 0:w],
                                    start=first, stop=last,
                                )
                                nc.tensor.matmul(
                                    pu[:, ch, 0:w],
                                    shifts[:, SH_S2mS1 if di == 1 else SH_S2mS0, :],
                                    up[:, ch, 0:w],
                                    start=first, stop=last,
                                )
                            con_i += 1
                        else:
                            last = con_i == n_con - 2
                            sh_neg = (SH_nS2, SH_nS1, SH_nS0)[di]
                            sh_pos = (SH_S2, SH_S1, SH_S0)[di]
                            for ch in range(c):
                                # direct: +w, +U at row offset 2, col 0
                                nc.tensor.matmul(
                                    pw[:, ch, 0:w],
                                    shifts[:, SH_S2, :],
                                    wp[:, ch, -cP : -cP + w],
                                    start=first, stop=False,
                                )
                                nc.tensor.matmul(
                                    pu[:, ch, 0:w],
                                    shifts[:, SH_S2, :],
                                    up[:, ch, -cP : -cP + w],
                                    start=first, stop=False,
                                )
                            for ch in range(c):
                                # shifted: +w, -U at row offset 2-di, col -dj
                                nc.tensor.matmul(
                                    pw[:, ch, 0:w],
                                    shifts[:, sh_pos, :],
                                    wp[:, ch, -dj - cP : -dj - cP + w],
                                    start=False, stop=last,
                                )
                                nc.tensor.matmul(
                                    pu[:, ch, 0:w],
                                    shifts[:, sh_neg, :],
                                    up[:, ch, -dj - cP : -dj - cP + w],
                                    start=False, stop=last,
                                )
                            con_i += 2

                # ---- finalize: out = p + pu * exp(-ln(pw + 1)) ----
                # High priority so the scheduler doesn't push it behind the
                # next tile's work (PSUM + ifs-buffer reuse gate on it).
                # pu is drained to SBUF by ACT early so both PSUM buffers
                # free ~3us after the last matmul instead of ~8us.
                with tc.high_priority(offset=250):
                    lnv = finals.tile([NOUT, c, w], f32, tag="lnv")
                    nc.scalar.activation(lnv[:], pw[:, :, 0:w], AF.Ln, bias=1.0)
                    puc = finals.tile([NOUT, c, w], f32, tag="puc")
                    nc.scalar.copy(puc[:], pu[:, :, 0:w])
                    rec = finals.tile([NOUT, c, w], f32, tag="rec")
                    nc.scalar.activation(rec[:], lnv[:], AF.Exp, scale=-1.0)
                    acct = finals.tile([NOUT, c, w], f32, tag="acct")
                    nc.vector.tensor_tensor(
                        acct[:], puc[:], rec[:], Alu.mult
                    )
                    res = finals.tile([NOUT, c, w], f32, tag="res")
                    nc.vector.tensor_tensor(
                        res[:], acct[:], ifs[2][0:NOUT, :, 2 : 2 + w], Alu.add
                    )
                    # overlap band: only write rows not already written
                    oo = 0 if r0 == bands[0] else max(0, prev_end - r0)
                    nc.sync.dma_start(
                        out_d[bi, :, r0 + oo : r0 + NOUT, :].rearrange(
                            "c r n -> r c n"
                        ),
                        res[oo:NOUT],
                    )
                prev_end = r0 + NOUT
    return nc


def _shift_mats():
    s = np.zeros((NG, N_SHIFT, NOUT), dtype=ml_dtypes.bfloat16)
    for k in range(3):
        for m in range(NOUT):
            s[m + k, k, m] = 1.0
    s[:, SH_S2pS1] = s[:, 2] + s[:, 1]
    s[:, SH_S2pS0] = s[:, 2] + s[:, 0]
    s[:, SH_nS0] = -s[:, 0]
    s[:, SH_nS1] = -s[:, 1]
    s[:, SH_nS2] = -s[:, 2]
    s[:, SH_S2mS1] = s[:, 2] - s[:, 1]
    s[:, SH_S2mS0] = s[:, 2] - s[:, 0]
    return s


def make_program(spatial_kernel):
    biases = {}
    for (di, dj) in PAIRS:
        v = float(np.float32(np.log(np.float32(spatial_kernel[2 + di, 2 + dj]))))
        biases[(di, dj)] = v
    nc = bacc.Bacc("TRN2", target_bir_lowering=False, debug=False)
    build_bilateral(nc, biases)
    nc.compile()
    return nc


def kernel(images, spatial_kernel):
    images = np.asarray(images, dtype=np.float32)
    spatial_kernel = np.asarray(spatial_kernel, dtype=np.float32)
    nc = make_program(spatial_kernel)
    shifts = _shift_mats()
    in_maps = [
        {"images": images[i * B_SH : (i + 1) * B_SH], "shifts": shifts}
        for i in range(N_CORES)
    ]
    res = run_bass_kernel_spmd(nc, in_maps, core_ids=list(range(N_CORES)))
    return np.concatenate([res.results[i]["out"] for i in range(N_CORES)], axis=0)


# revision 12
# speedup vs baseline: 1.1282x; 1.1282x over previous
"""Bilateral anti-alias filter on Trainium2, 8-core data parallel.

Full inputs: images [16,3,512,512] f32, spatial_kernel [5,5] f32.
Shards the batch over 8 NeuronCores (2 images each), runs a Bass/Tile
kernel per core, gathers the full output.

Math (per pixel, K=5, sigma_i=0.1):
  w_t = s_t * exp(-50*(p_t - c)^2),  out = sum(w_t*p_t)/(sum(w_t)+eps)
Pair symmetry: w for tap -(di,dj) at pixel r equals w for tap +(di,dj)
at pixel r-(di,dj), so only 12 intensity planes e_v = exp(-50 d_v^2)
are computed (d_v = p(.+v) - p).

U-trick: with U_v = e_v*d_v,
  num = p*den + sum_v s_v*[U_v(r) - U_v(r-v)]  =>  out = p + pu/(1+pw)
so per pair only ONE product (U) is needed on the Vector engine. The
spatial weights s_v, the +/-1 signs and the row shifts are all folded
into small bf16 matrices fed to the TensorEngine, which accumulates
pw (denominator-1) and pu (numerator correction) into PSUM. Column
shifts are SBUF slices (free).

Structure per core: 2 images x 4 full 124-row bands + ONE combined
tile holding BOTH images' last 16 rows as two 20-partition segments
(contraction K=40) -- 9 tiles instead of 10.

Engine notes: supergroups of 4 tap-pairs halve ACT/DVE instruction
and semaphore counts (possible because s_v lives in the matmul
weights, not the exp bias). GpSimd is kept idle: it shares an
exclusive SBUF port pair with the DVE's second read port, so any
GpSimd op blocks every tensor_tensor.
"""
import sys

sys.path.insert(0, "/opt/trn_rl_repo")

import os
import numpy as np
import ml_dtypes
from contextlib import ExitStack, nullcontext

import concourse.bass as bass
import concourse.tile as tile
from concourse import bacc, mybir
from concourse.bass_utils import run_bass_kernel_spmd

f32 = mybir.dt.float32
bf16 = mybir.dt.bfloat16
AF = mybir.ActivationFunctionType
Alu = mybir.AluOpType

N_CORES = 8
B_FULL, C, H, W = 16, 3, 512, 512
B_SH = B_FULL // N_CORES  # 2 images per core
INV2SIG2 = 1.0 / (2.0 * 0.1 * 0.1)  # 50.0

# 12 pairs (di, dj) with di >= 0, lexicographically positive
PAIRS = [
    (0, 1), (0, 2),
    (1, -2), (1, -1), (1, 0), (1, 1), (1, 2),
    (2, -2), (2, -1), (2, 0), (2, 1), (2, 2),
]

# supergroups of 4 pairs (s folded into lhsT, so grouping is free)
GROUPS = [
    [(0, 1), (1, 0), (1, -1), (1, 1)],
    [(0, 2), (2, 0), (1, -2), (1, 2)],
    [(2, -1), (2, 1), (2, -2), (2, 2)],
]
SQ_DVE_GROUPS = {int(x) for x in os.environ.get("K_SQDVE", "1").split(",") if x != ""}
NOUT = 124   # output rows per full tile
NG = 128     # grid partitions (= NOUT + 4)
WB = W + 4   # 516: padded col buffer, idx j <-> col j-2
NSEG = 16    # output rows per combo-tile segment

HP_FIN = int(os.environ.get("K_HP", "1"))
HP_OFF = int(os.environ.get("K_HPOFF", "250"))
BUFS_F = int(os.environ.get("K_BUFSF", "2"))
BUFS_B = int(os.environ.get("K_BUFSB", "2"))

# tile type 0: one 124-row band at partition 0
# tile type 1: two (16+4)-row segments (both images' last 16 rows)
#   (p0, q0, nout): partition base, psum-row base, output rows
SEG_LAYOUTS = [
    [(0, 0, NOUT)],
    [(0, 0, NSEG), (NSEG + 4, NSEG + 4, NSEG)],
]


def _vkey(kind, di, dj):
    d2 = di * di + dj * dj
    if kind == "dir":
        return ("S", 2, d2)
    if kind == "pos":
        return ("S", 2 - di, d2)
    if kind == "neg":
        return ("N", 2 - di, d2)
    return ("MW" if kind == "mrgw" else "MU", di, d2)


def _variant_keys():
    keys = []
    for (di, dj) in PAIRS:
        kinds = ("mrgw", "mrgu") if dj == 0 else ("dir", "pos", "neg")
        for kind in kinds:
            t = _vkey(kind, di, dj)
            if t not in keys:
                keys.append(t)
    return keys


VKEYS = _variant_keys()
NV = len(VKEYS)


def _s_of_d2(spatial):
    m = {}
    for (di, dj) in PAIRS:
        m.setdefault(di * di + dj * dj, spatial[2 + di, 2 + dj])
    return m


def _shift_mats(spatial):
    """lhsT bank [NG, 2 tile-types, NV variants, NOUT] bf16.
    Row shifts, +/- signs and spatial weights baked together."""
    spatial = np.asarray(spatial, np.float32)
    sd2 = _s_of_d2(spatial)
    arr = np.zeros((NG, 2, NV, NOUT), np.float32)
    for tt, segs in enumerate(SEG_LAYOUTS):
        for vi, (kind, kk, d2) in enumerate(VKEYS):
            s = sd2[d2]
            for (p0, q0, n) in segs:
                for m in range(n):
                    o = q0 + m
                    if kind == "S":
                        arr[p0 + m + kk, tt, vi, o] += s
                    elif kind == "N":
                        arr[p0 + m + kk, tt, vi, o] -= s
                    elif kind == "MW":
                        arr[p0 + m + 2, tt, vi, o] += s
                        arr[p0 + m + 2 - kk, tt, vi, o] += s
                    elif kind == "MU":
                        arr[p0 + m + 2, tt, vi, o] += s
                        arr[p0 + m + 2 - kk, tt, vi, o] -= s
    return arr.astype(ml_dtypes.bfloat16)


def _reflect_runs(v0, v1, h):
    """Split virtual row range [v0, v1] into runs of physical rows.
    Returns list of (p_offset, phys_start, count, step) with step +-1."""
    runs = []
    v = v0
    while v <= v1:
        if v < 0:
            e = min(-1, v1)
            runs.append((v - v0, -v, e - v + 1, -1))
            v = e + 1
        elif v >= h:
            e = v1
            runs.append((v - v0, 2 * h - 2 - v, e - v + 1, -1))
            v = e + 1
        else:
            e = min(h - 1, v1)
            runs.append((v - v0, v, e - v + 1, 1))
            v = e + 1
    return runs


def build_bilateral(nc, h=H, w=W, b_sh=B_SH, c=C):
    """Emit the per-core program into nc (a Bacc)."""
    wb = w + 4
    img_d = nc.dram_tensor("images", [b_sh, c, h, w], f32, kind="ExternalInput").ap()
    shifts_d = nc.dram_tensor(
        "shifts", [NG, 2, NV, NOUT], bf16, kind="ExternalInput"
    ).ap()
    out_d = nc.dram_tensor("out", [b_sh, c, h, w], f32, kind="ExternalOutput").ap()

    for val in (0.0, 1.0):
        key = (f32, val)
        if key not in nc.const_aps.aps:
            t = nc.alloc_sbuf_tensor(f"cbias-{val}", [128, 1], f32)
            nc.gpsimd.memset(t.ap(), val)
            nc.const_aps.aps[key] = t.ap()
    nc.all_engine_barrier()

    # One ACT table set serves Square+Exp+Ln; preload it once so the
    # compiler's greedy per-function chooser doesn't thrash.
    from concourse.hw_specs import get_activation_tables

    set_names = list(get_activation_tables(nc.m.arch))
    nc.scalar.add_instruction(
        mybir.InstLoadActFuncSet(
            name=nc.get_next_instruction_name(),
            act_func_set_id=set_names.index("natural_log_exp_and_others"),
            ins=[],
            outs=[],
        )
    )

    # tile list: (tile_type, [(p0, q0, nout, r0, bi), ...])
    tiles_l = []
    for bi in range(b_sh):
        for r0 in range(0, h - NOUT - NSEG + 1, NOUT):
            tiles_l.append((0, [(0, 0, NOUT, r0, bi)]))
    tiles_l.append(
        (1, [(p0, q0, n, h - NSEG, si)
             for si, (p0, q0, n) in enumerate(SEG_LAYOUTS[1])])
    )

    with tile.TileContext(nc) as tc, ExitStack() as ctx:
        consts = ctx.enter_context(tc.tile_pool(name="consts", bufs=1))
        imgs_f = ctx.enter_context(tc.tile_pool(name="imgs_f", bufs=BUFS_F))
        imgs_b = ctx.enter_context(tc.tile_pool(name="imgs_b", bufs=BUFS_B))
        planes = ctx.enter_context(tc.tile_pool(name="planes", bufs=2))
        prods = ctx.enter_context(tc.tile_pool(name="prods", bufs=2))
        finals = ctx.enter_context(tc.tile_pool(name="finals", bufs=1))
        psums = ctx.enter_context(tc.tile_pool(name="psums", bufs=1, space="PSUM"))

        shifts = consts.tile([NG, 2, NV, NOUT], bf16)
        nc.sync.dma_start(shifts[:], shifts_d[:])

        def load_ifs(segs):
            """DMA loads of the 3 row-shifted f32 copies for all segments.
            Issued one tile ahead of its compute."""
            ifs = []
            for s in range(3):
                t = imgs_f.tile([NG, c, wb], f32, tag=f"i{s}f")
                for (p0, q0, n, r0, bi) in segs:
                    refl_rows = []
                    for (po, ps, cnt, step) in _reflect_runs(
                        r0 - 2 + s, r0 - 2 + s + (n + 4) - 1, h
                    ):
                        if step == 1:
                            src = img_d[bi, :, ps : ps + cnt, :]
                            nc.sync.dma_start(
                                t[p0 + po : p0 + po + cnt, :, 2 : 2 + w],
                                src.rearrange("c r n -> r c n"),
                            )
                        else:
                            for k in range(cnt):
                                refl_rows.append((p0 + po + k, ps - k))
                    for (p_dst, phys) in refl_rows:
                        p_src = p0 + phys - (r0 - 2 + s)
                        nc.sync.dma_start(
                            t[p_dst : p_dst + 1, :, 2 : 2 + w],
                            t[p_src : p_src + 1, :, 2 : 2 + w],
                        )
                ifs.append(t)
            return ifs

        pending = load_ifs(tiles_l[0][1])
        for t_i, (tt, segs) in enumerate(tiles_l):
            ifs = pending
            if t_i + 1 < len(tiles_l):
                pending = load_ifs(tiles_l[t_i + 1][1])

            ngt = segs[-1][0] + segs[-1][2] + 4  # used partitions
            npo = segs[-1][1] + segs[-1][2]      # psum output rows

            # reflect pad cols: tiny DVE copies (NOT GpSimd: GpSimd ops
            # block the shared SBUF port the DVE TTs need)
            for t in ifs:
                for (j, jsrc) in ((0, 4), (1, 3), (2 + w, w), (3 + w, w - 1)):
                    nc.vector.tensor_copy(
                        t[0:ngt, :, j : j + 1], t[0:ngt, :, jsrc : jsrc + 1]
                    )

            # bf16 copies: A (cast), B (A shifted 1 col, via DMA)
            ibA, ibB = [], []
            for s in range(3):
                a = imgs_b.tile([NG, c, wb], bf16, tag=f"i{s}bA")
                nc.vector.tensor_copy(a[0:ngt], ifs[s][0:ngt])
                ibA.append(a)
                b = imgs_b.tile([NG, c, wb], bf16, tag=f"i{s}bB")
                nc.sync.dma_start(b[0:ngt, :, 0 : wb - 1], a[0:ngt, :, 1:wb])
                ibB.append(b)

            # ---- PSUM accumulators ----
            pw = psums.tile([NOUT, c, 512], f32, tag="pw")
            pu = psums.tile([NOUT, c, 512], f32, tag="pu")

            n_con = 2 * len(PAIRS) - 2  # pw/pu contributions per channel
            con_i = 0
            for g_i, grp in enumerate(GROUPS):
                G = len(grp)
                dg = planes.tile([NG, G * c, w + 2], bf16, tag="d")
                for gi, (di, dj) in enumerate(grp):
                    cP = -2 if dj > 0 else 0
                    if dj % 2 == 0:
                        dsrc = ibA[di][0:ngt, :, cP + dj + 2 : cP + dj + 4 + w]
                    else:
                        dsrc = ibB[di][0:ngt, :, cP + dj + 1 : cP + dj + 3 + w]
                    nc.vector.tensor_tensor(
                        dg[0:ngt, gi * c : (gi + 1) * c, :], dsrc,
                        ibA[0][0:ngt, :, cP + 2 : cP + 4 + w], Alu.subtract,
                    )
                sqg = planes.tile([NG, G * c, w + 2], bf16, tag="sq")
                if g_i in SQ_DVE_GROUPS:
                    nc.vector.tensor_tensor(
                        sqg[0:ngt], dg[0:ngt], dg[0:ngt], Alu.mult
                    )
                else:
                    nc.scalar.activation(sqg[0:ngt], dg[0:ngt], AF.Square)
                wg = planes.tile([NG, G * c, w + 2], bf16, tag="w")
                nc.scalar.activation(
                    wg[0:ngt], sqg[0:ngt], AF.Exp, bias=0.0, scale=-INV2SIG2
                )
                ug = prods.tile([NG, G * c, w + 2], bf16, tag="u")
                nc.vector.tensor_tensor(
                    ug[0:ngt], wg[0:ngt], dg[0:ngt], Alu.mult
                )

                for gi, (di, dj) in enumerate(grp):
                    cP = -2 if dj > 0 else 0
                    wp = wg[0:ngt, gi * c : (gi + 1) * c, :]
                    up = ug[0:ngt, gi * c : (gi + 1) * c, :]
                    first = con_i == 0

                    def lhsT(kind):
                        vi = VKEYS.index(_vkey(kind, di, dj))
                        return shifts[0:ngt, tt, vi, 0:npo]

                    if dj == 0:
                        last = con_i == n_con - 1
                        for ch in range(c):
                            nc.tensor.matmul(
                                pw[0:npo, ch, 0:w], lhsT("mrgw"),
                                wp[:, ch, 0:w], start=first, stop=last,
                            )
                            nc.tensor.matmul(
                                pu[0:npo, ch, 0:w], lhsT("mrgu"),
                                up[:, ch, 0:w], start=first, stop=last,
                            )
                        con_i += 1
                    else:
                        last = con_i == n_con - 2
                        for ch in range(c):
                            nc.tensor.matmul(
                                pw[0:npo, ch, 0:w], lhsT("dir"),
                                wp[:, ch, -cP : -cP + w],
                                start=first, stop=False,
                            )
                            nc.tensor.matmul(
                                pu[0:npo, ch, 0:w], lhsT("dir"),
                                up[:, ch, -cP : -cP + w],
                                start=first, stop=False,
                            )
                        for ch in range(c):
                            nc.tensor.matmul(
                                pw[0:npo, ch, 0:w], lhsT("pos"),
                                wp[:, ch, -dj - cP : -dj - cP + w],
                                start=False, stop=last,
                            )
                            nc.tensor.matmul(
                                pu[0:npo, ch, 0:w], lhsT("neg"),
                                up[:, ch, -dj - cP : -dj - cP + w],
                                start=False, stop=last,
                            )
                        con_i += 2

            # ---- finalize: out = p + pu * exp(-ln(pw + 1)) ----
            with (tc.high_priority(offset=HP_OFF) if HP_FIN else nullcontext()):
                lnv = finals.tile([NOUT, c, w], f32, tag="lnv")
                nc.scalar.activation(lnv[0:npo], pw[0:npo, :, 0:w], AF.Ln, bias=1.0)
                rec = finals.tile([NOUT, c, w], f32, tag="rec")
                nc.scalar.activation(rec[0:npo], lnv[0:npo], AF.Exp, scale=-1.0)
                acct = finals.tile([NOUT, c, w], f32, tag="acct")
                nc.vector.tensor_tensor(
                    acct[0:npo], pu[0:npo, :, 0:w], rec[0:npo], Alu.mult
                )
                res = finals.tile([NOUT, c, w], f32, tag="res")
                nc.vector.tensor_tensor(
                    res[0:npo], acct[0:npo], ifs[2][0:npo, :, 2 : 2 + w], Alu.add
                )
                for (p0, q0, n, r0, bi) in segs:
                    nc.sync.dma_start(
                        out_d[bi, :, r0 : r0 + n, :].rearrange("c r n -> r c n"),
                        res[q0 : q0 + n],
                    )
    return nc


def make_program(spatial_kernel=None):
    nc = bacc.Bacc("TRN2", target_bir_lowering=False, debug=False)
    build_bilateral(nc)
    nc.compile()
    return nc


def kernel(images, spatial_kernel):
    images = np.asarray(images, dtype=np.float32)
    spatial_kernel = np.asarray(spatial_kernel, dtype=np.float32)
    nc = make_program()
    shifts = _shift_mats(spatial_kernel)
    in_maps = [
        {"images": images[i * B_SH : (i + 1) * B_SH], "shifts": shifts}
        for i in range(N_CORES)
    ]
    res = run_bass_kernel_spmd(nc, in_maps, core_ids=list(range(N_CORES)))
    return np.concatenate([res.results[i]["out"] for i in range(N_CORES)], axis=0)


# revision 13
# speedup vs baseline: 1.1522x; 1.0213x over previous
"""Bilateral anti-alias filter on Trainium2, 8-core data parallel.

Full inputs: images [16,3,512,512] f32, spatial_kernel [5,5] f32.
Shards the batch over 8 NeuronCores (2 images each), runs a Bass/Tile
kernel per core, gathers the full output.

Math (per pixel, K=5, sigma_i=0.1):
  w_t = s_t * exp(-50*(p_t - c)^2),  out = sum(w_t*p_t)/(sum(w_t)+eps)
Pair symmetry: w for tap -(di,dj) at pixel r equals w for tap +(di,dj)
at pixel r-(di,dj), so only 12 intensity planes e_v = exp(-50 d_v^2)
are computed (d_v = p(.+v) - p).

U-trick: with U_v = e_v*d_v,
  num = p*den + sum_v s_v*[U_v(r) - U_v(r-v)]  =>  out = p + pu/(1+pw)
so per pair only ONE product (U) is needed on the Vector engine. The
spatial weights s_v, the +/-1 signs and the row shifts are all folded
into small bf16 matrices fed to the TensorEngine, which accumulates
pw (denominator-1) and pu (numerator correction) into PSUM. Column
shifts are SBUF slices (free).

Structure per core: 2 images x 4 full 124-row bands + ONE combined
tile holding BOTH images' last 16 rows as two 20-partition segments
(contraction K=40) -- 9 tiles instead of 10.

Engine notes: supergroups of 4 tap-pairs halve ACT/DVE instruction
and semaphore counts (possible because s_v lives in the matmul
weights, not the exp bias). GpSimd is kept idle: it shares an
exclusive SBUF port pair with the DVE's second read port, so any
GpSimd op blocks every tensor_tensor.
"""
import sys

sys.path.insert(0, "/opt/trn_rl_repo")

import os
import numpy as np
import ml_dtypes
from contextlib import ExitStack, nullcontext

import concourse.bass as bass
import concourse.tile as tile
from concourse import bacc, mybir
from concourse.bass_utils import run_bass_kernel_spmd

f32 = mybir.dt.float32
bf16 = mybir.dt.bfloat16
AF = mybir.ActivationFunctionType
Alu = mybir.AluOpType

N_CORES = 8
B_FULL, C, H, W = 16, 3, 512, 512
B_SH = B_FULL // N_CORES  # 2 images per core
INV2SIG2 = 1.0 / (2.0 * 0.1 * 0.1)  # 50.0

# 12 pairs (di, dj) with di >= 0, lexicographically positive
PAIRS = [
    (0, 1), (0, 2),
    (1, -2), (1, -1), (1, 0), (1, 1), (1, 2),
    (2, -2), (2, -1), (2, 0), (2, 1), (2, 2),
]

# supergroups of 4 pairs (s folded into lhsT, so grouping is free)
GROUPS = [
    [(0, 1), (1, 0), (1, -1), (1, 1)],
    [(0, 2), (2, 0), (1, -2), (1, 2)],
    [(2, -1), (2, 1), (2, -2), (2, 2)],
]
SQ_DVE_GROUPS = {int(x) for x in os.environ.get("K_SQDVE", "1").split(",") if x != ""}
NOUT = 124   # output rows per full tile
NG = 128     # grid partitions (= NOUT + 4)
WB = W + 4   # 516: padded col buffer, idx j <-> col j-2
NSEG = 16    # output rows per combo-tile segment

HP_FIN = int(os.environ.get("K_HP", "1"))
HP_OFF = int(os.environ.get("K_HPOFF", "250"))
BUFS_F = int(os.environ.get("K_BUFSF", "2"))
BUFS_B = int(os.environ.get("K_BUFSB", "2"))

# tile type 0: one 124-row band at partition 0
# tile type 1: two (16+4)-row segments (both images' last 16 rows)
#   (p0, q0, nout): partition base, psum-row base, output rows
SEG_LAYOUTS = [
    [(0, 0, NOUT)],
    [(0, 0, NSEG), (NSEG + 4, NSEG + 4, NSEG)],
]


def _vkey(kind, di, dj):
    d2 = di * di + dj * dj
    if kind == "dir":
        return ("S", 2, d2)
    if kind == "pos":
        return ("S", 2 - di, d2)
    if kind == "neg":
        return ("N", 2 - di, d2)
    return ("MW" if kind == "mrgw" else "MU", di, d2)


def _variant_keys():
    keys = []
    for (di, dj) in PAIRS:
        kinds = ("mrgw", "mrgu") if dj == 0 else ("dir", "pos", "neg")
        for kind in kinds:
            t = _vkey(kind, di, dj)
            if t not in keys:
                keys.append(t)
    return keys


VKEYS = _variant_keys()
NV = len(VKEYS)


def _s_of_d2(spatial):
    m = {}
    for (di, dj) in PAIRS:
        m.setdefault(di * di + dj * dj, spatial[2 + di, 2 + dj])
    return m


def _shift_mats(spatial):
    """lhsT bank [NG, 2 tile-types, NV variants, NOUT] bf16.
    Row shifts, +/- signs and spatial weights baked together."""
    spatial = np.asarray(spatial, np.float32)
    sd2 = _s_of_d2(spatial)
    arr = np.zeros((NG, 2, NV, NOUT), np.float32)
    for tt, segs in enumerate(SEG_LAYOUTS):
        for vi, (kind, kk, d2) in enumerate(VKEYS):
            s = sd2[d2]
            for (p0, q0, n) in segs:
                for m in range(n):
                    o = q0 + m
                    if kind == "S":
                        arr[p0 + m + kk, tt, vi, o] += s
                    elif kind == "N":
                        arr[p0 + m + kk, tt, vi, o] -= s
                    elif kind == "MW":
                        arr[p0 + m + 2, tt, vi, o] += s
                        arr[p0 + m + 2 - kk, tt, vi, o] += s
                    elif kind == "MU":
                        arr[p0 + m + 2, tt, vi, o] += s
                        arr[p0 + m + 2 - kk, tt, vi, o] -= s
    return arr.astype(ml_dtypes.bfloat16)


def _reflect_runs(v0, v1, h):
    """Split virtual row range [v0, v1] into runs of physical rows.
    Returns list of (p_offset, phys_start, count, step) with step +-1."""
    runs = []
    v = v0
    while v <= v1:
        if v < 0:
            e = min(-1, v1)
            runs.append((v - v0, -v, e - v + 1, -1))
            v = e + 1
        elif v >= h:
            e = v1
            runs.append((v - v0, 2 * h - 2 - v, e - v + 1, -1))
            v = e + 1
        else:
            e = min(h - 1, v1)
            runs.append((v - v0, v, e - v + 1, 1))
            v = e + 1
    return runs


def build_bilateral(nc, h=H, w=W, b_sh=B_SH, c=C):
    """Emit the per-core program into nc (a Bacc)."""
    wb = w + 4
    img_d = nc.dram_tensor("images", [b_sh, c, h, w], f32, kind="ExternalInput").ap()
    shifts_d = nc.dram_tensor(
        "shifts", [NG, 2, NV, NOUT], bf16, kind="ExternalInput"
    ).ap()
    out_d = nc.dram_tensor("out", [b_sh, c, h, w], f32, kind="ExternalOutput").ap()

    for val in (0.0, 1.0):
        key = (f32, val)
        if key not in nc.const_aps.aps:
            t = nc.alloc_sbuf_tensor(f"cbias-{val}", [128, 1], f32)
            nc.gpsimd.memset(t.ap(), val)
            nc.const_aps.aps[key] = t.ap()
    nc.all_engine_barrier()

    # One ACT table set serves Square+Exp+Ln; preload it once so the
    # compiler's greedy per-function chooser doesn't thrash.
    from concourse.hw_specs import get_activation_tables

    set_names = list(get_activation_tables(nc.m.arch))
    nc.scalar.add_instruction(
        mybir.InstLoadActFuncSet(
            name=nc.get_next_instruction_name(),
            act_func_set_id=set_names.index("natural_log_exp_and_others"),
            ins=[],
            outs=[],
        )
    )

    # tile list: (tile_type, [(p0, q0, nout, r0, bi), ...])
    tiles_l = []
    for bi in range(b_sh):
        for r0 in range(0, h - NOUT - NSEG + 1, NOUT):
            tiles_l.append((0, [(0, 0, NOUT, r0, bi)]))
    tiles_l.append(
        (1, [(p0, q0, n, h - NSEG, si)
             for si, (p0, q0, n) in enumerate(SEG_LAYOUTS[1])])
    )

    with tile.TileContext(nc) as tc, ExitStack() as ctx:
        consts = ctx.enter_context(tc.tile_pool(name="consts", bufs=1))
        imgs_f = ctx.enter_context(tc.tile_pool(name="imgs_f", bufs=BUFS_F))
        imgs_b = ctx.enter_context(tc.tile_pool(name="imgs_b", bufs=BUFS_B))
        planes = ctx.enter_context(tc.tile_pool(name="planes", bufs=2))
        prods = ctx.enter_context(tc.tile_pool(name="prods", bufs=2))
        finals = ctx.enter_context(tc.tile_pool(name="finals", bufs=1))
        psums = ctx.enter_context(tc.tile_pool(name="psums", bufs=1, space="PSUM"))

        shifts = consts.tile([NG, 2, NV, NOUT], bf16)
        nc.sync.dma_start(shifts[:], shifts_d[:])

        def load_ifs(segs):
            """DMA loads of the 3 row-shifted f32 copies for all segments.
            Issued one tile ahead of its compute."""
            ifs = []
            for s in range(3):
                t = imgs_f.tile([NG, c, wb], f32, tag=f"i{s}f")
                for (p0, q0, n, r0, bi) in segs:
                    refl_rows = []
                    for (po, ps, cnt, step) in _reflect_runs(
                        r0 - 2 + s, r0 - 2 + s + (n + 4) - 1, h
                    ):
                        if step == 1:
                            src = img_d[bi, :, ps : ps + cnt, :]
                            nc.sync.dma_start(
                                t[p0 + po : p0 + po + cnt, :, 2 : 2 + w],
                                src.rearrange("c r n -> r c n"),
                            )
                        else:
                            for k in range(cnt):
                                refl_rows.append((p0 + po + k, ps - k))
                    for (p_dst, phys) in refl_rows:
                        p_src = p0 + phys - (r0 - 2 + s)
                        nc.sync.dma_start(
                            t[p_dst : p_dst + 1, :, 2 : 2 + w],
                            t[p_src : p_src + 1, :, 2 : 2 + w],
                        )
                ifs.append(t)
            return ifs

        pending = load_ifs(tiles_l[0][1])
        for t_i, (tt, segs) in enumerate(tiles_l):
            ifs = pending
            if t_i + 1 < len(tiles_l):
                pending = load_ifs(tiles_l[t_i + 1][1])

            ngt = segs[-1][0] + segs[-1][2] + 4  # used partitions
            npo = segs[-1][1] + segs[-1][2]      # psum output rows

            # reflect pad cols: tiny DVE copies (NOT GpSimd: GpSimd ops
            # block the shared SBUF port the DVE TTs need)
            for t in ifs:
                for (j, jsrc) in ((0, 4), (1, 3), (2 + w, w), (3 + w, w - 1)):
                    nc.vector.tensor_copy(
                        t[0:ngt, :, j : j + 1], t[0:ngt, :, jsrc : jsrc + 1]
                    )

            # bf16 copies: A (cast), B (A shifted 1 col, via DMA)
            ibA, ibB = [], []
            for s in range(3):
                a = imgs_b.tile([NG, c, wb], bf16, tag=f"i{s}bA")
                nc.vector.tensor_copy(a[0:ngt], ifs[s][0:ngt])
                ibA.append(a)
                b = imgs_b.tile([NG, c, wb], bf16, tag=f"i{s}bB")
                nc.vector.tensor_copy(b[0:ngt, :, 0 : wb - 1], a[0:ngt, :, 1:wb])
                ibB.append(b)

            # ---- PSUM accumulators ----
            pw = psums.tile([NOUT, c, 512], f32, tag="pw")
            pu = psums.tile([NOUT, c, 512], f32, tag="pu")

            n_con = 2 * len(PAIRS) - 2  # pw/pu contributions per channel
            con_i = 0
            for g_i, grp in enumerate(GROUPS):
                G = len(grp)
                dg = planes.tile([NG, G * c, w + 2], bf16, tag="d")
                for gi, (di, dj) in enumerate(grp):
                    cP = -2 if dj > 0 else 0
                    if dj % 2 == 0:
                        dsrc = ibA[di][0:ngt, :, cP + dj + 2 : cP + dj + 4 + w]
                    else:
                        dsrc = ibB[di][0:ngt, :, cP + dj + 1 : cP + dj + 3 + w]
                    nc.vector.tensor_tensor(
                        dg[0:ngt, gi * c : (gi + 1) * c, :], dsrc,
                        ibA[0][0:ngt, :, cP + 2 : cP + 4 + w], Alu.subtract,
                    )
                sqg = planes.tile([NG, G * c, w + 2], bf16, tag="sq")
                if g_i in SQ_DVE_GROUPS:
                    nc.vector.tensor_tensor(
                        sqg[0:ngt], dg[0:ngt], dg[0:ngt], Alu.mult
                    )
                else:
                    nc.scalar.activation(sqg[0:ngt], dg[0:ngt], AF.Square)
                wg = planes.tile([NG, G * c, w + 2], bf16, tag="w")
                nc.scalar.activation(
                    wg[0:ngt], sqg[0:ngt], AF.Exp, bias=0.0, scale=-INV2SIG2
                )
                ug = prods.tile([NG, G * c, w + 2], bf16, tag="u")
                nc.vector.tensor_tensor(
                    ug[0:ngt], wg[0:ngt], dg[0:ngt], Alu.mult
                )

                for gi, (di, dj) in enumerate(grp):
                    cP = -2 if dj > 0 else 0
                    wp = wg[0:ngt, gi * c : (gi + 1) * c, :]
                    up = ug[0:ngt, gi * c : (gi + 1) * c, :]
                    first = con_i == 0

                    def lhsT(kind):
                        vi = VKEYS.index(_vkey(kind, di, dj))
                        return shifts[0:ngt, tt, vi, 0:npo]

                    if dj == 0:
                        last = con_i == n_con - 1
                        for ch in range(c):
                            nc.tensor.matmul(
                                pw[0:npo, ch, 0:w], lhsT("mrgw"),
                                wp[:, ch, 0:w], start=first, stop=last,
                            )
                            nc.tensor.matmul(
                                pu[0:npo, ch, 0:w], lhsT("mrgu"),
                                up[:, ch, 0:w], start=first, stop=last,
                            )
                        con_i += 1
                    else:
                        last = con_i == n_con - 2
                        for ch in range(c):
                            nc.tensor.matmul(
                                pw[0:npo, ch, 0:w], lhsT("dir"),
                                wp[:, ch, -cP : -cP + w],
                                start=first, stop=False,
                            )
                            nc.tensor.matmul(
                                pu[0:npo, ch, 0:w], lhsT("dir"),
                                up[:, ch, -cP : -cP + w],
                                start=first, stop=False,
                            )
                        for ch in range(c):
                            nc.tensor.matmul(
                                pw[0:npo, ch, 0:w], lhsT("pos"),
                                wp[:, ch, -dj - cP : -dj - cP + w],
                                start=False, stop=last,
                            )
                            nc.tensor.matmul(
                                pu[0:npo, ch, 0:w], lhsT("neg"),
                                up[:, ch, -dj - cP : -dj - cP + w],
                                start=False, stop=last,
                            )
                        con_i += 2

            # ---- finalize: out = p + pu * exp(-ln(pw + 1)) ----
            with (tc.high_priority(offset=HP_OFF) if HP_FIN else nullcontext()):
                lnv = finals.tile([NOUT, c, w], f32, tag="lnv")
                nc.scalar.activation(lnv[0:npo], pw[0:npo, :, 0:w], AF.Ln, bias=1.0)
                rec = finals.tile([NOUT, c, w], f32, tag="rec")
                nc.scalar.activation(rec[0:npo], lnv[0:npo], AF.Exp, scale=-1.0)
                acct = finals.tile([NOUT, c, w], f32, tag="acct")
                nc.vector.tensor_tensor(
                    acct[0:npo], pu[0:npo, :, 0:w], rec[0:npo], Alu.mult
                )
                res = finals.tile([NOUT, c, w], f32, tag="res")
                nc.vector.tensor_tensor(
                    res[0:npo], acct[0:npo], ifs[2][0:npo, :, 2 : 2 + w], Alu.add
                )
                for (p0, q0, n, r0, bi) in segs:
                    nc.sync.dma_start(
                        out_d[bi, :, r0 : r0 + n, :].rearrange("c r n -> r c n"),
                        res[q0 : q0 + n],
                    )
    return nc


def make_program(spatial_kernel=None):
    nc = bacc.Bacc("TRN2", target_bir_lowering=False, debug=False)
    build_bilateral(nc)
    nc.compile()
    return nc


def kernel(images, spatial_kernel):
    images = np.asarray(images, dtype=np.float32)
    spatial_kernel = np.asarray(spatial_kernel, dtype=np.float32)
    nc = make_program()
    shifts = _shift_mats(spatial_kernel)
    in_maps = [
        {"images": images[i * B_SH : (i + 1) * B_SH], "shifts": shifts}
        for i in range(N_CORES)
    ]
    res = run_bass_kernel_spmd(nc, in_maps, core_ids=list(range(N_CORES)))
    return np.concatenate([res.results[i]["out"] for i in range(N_CORES)], axis=0)
